# revision 49
# baseline (speedup 1.0000x reference)
"""Causal self-attention (B=2, T=2048, C=1024, nh=16) on 8 TRN2 NeuronCores.

Sharding: core c -> batch b = c//4, head group g = c%4 (4 heads each).
Each core computes QKV projections for its heads, causal attention, and a
partial output projection (W_proj rows for its heads). The four partials per
batch are summed on the host, which also adds b_proj.

Layouts (per core, hardcoded):
  xt    [128, 8, 2048]     x[b].T tiles:  xt[p, kt, t] = x[b, t, kt*128+p]
  wqkv  [128, 8, 6, 128]   W_attn q|k|v cols for this core's heads
  bqk   [128, 4] f32       b_attn q|k (per-partition bias)
  bv    [1, 256] f32       b_attn v (broadcast on device)
  wp    [128, 2, 1024]     W_proj rows for this core's heads
  out   [2048, 1024] bf16  partial (x[b] @ ... for this head group)

In-kernel dataflow (all matmuls bf16 with fp32 PSUM accumulation):
  qT,kT = (W.T @ x.T)      [feat, t] layout  (lhsT=W tile, rhs=xT)
  v     = (x @ Wv)         [t, feat] layout, written into vext (65-col per
                           head, ones column -> l rides along in PV)
  S^T   = k @ q.T          [j, i] layout. The contraction is only hs=64, so
          the two heads of a pair run CONCURRENTLY as 64-row-tiled matmuls
          (tile_position (0,0) and (64,0)): head A's k/q feats live on
          partitions 0-63, head B's on 64-127. One j-tile costs ~N cycles
          for BOTH heads instead of 2N with the old zero-padded K=128 form.
  P^T   = exp(S^T/8), masked on diagonal tiles (mult by 0/1 triangle)
  y^T,l = [v|1].T @ P^T    [d, i] layout, row 64 = l = sum_j P  (128x128)
  yT    = y^T * (1/l broadcast)
  out   = yT.T @ Wp        (bias added on host)

Schedule: phase-1 for tb=0 runs up front; attention i-block ib then runs
with deferred work braided into its exp-paced windows. ib=3 is ACT-bound
(~34us of exp vs ~25us of its own S+PV), so tb=3's k/v projections and the
proj blocks of ib=1,2 are deferred into it.
"""

import os
import sys

sys.path.insert(0, "/opt/trn_rl_repo")
os.environ.setdefault("MYCRO_LOCAL_CACHE", "1")

import ml_dtypes
import numpy as np

import concourse.bass as bass
import concourse.mybir as mybir
import concourse.tile as tile
from concourse import bacc
from concourse.bass_utils import run_bass_kernel_spmd

B, T, C, NH, HS = 2, 2048, 1024, 16, 64
HPC = 4  # heads per core
N_CORES = 8
KT = C // 128  # 8 contraction tiles over C
TT = T // 128  # 16 tiles over T
IB = T // 512  # 4 i-blocks over T
F32 = mybir.dt.float32

CD = mybir.dt.bfloat16
CD_NP = ml_dtypes.bfloat16

LAST_RESULT = None
_CACHE = {}


def _emit(nc, tc, ctx, aps):
    xt, wqkv, bqk, bv, wp, out = (
        aps["xt"], aps["wqkv"], aps["bqk"], aps["bv"], aps["wp"], aps["out"],
    )
    Exp = mybir.ActivationFunctionType.Exp

    consts = ctx.enter_context(tc.tile_pool(name="consts", bufs=1))

    dummy = consts.tile([128, 512], CD, tag="dummy")
    nc.vector.memset(dummy[:], 1.0)

    # --- persistent SBUF tensors. First-wave DMAs (x tb=0 + weights) are
    # 2-kt pieces split between the sync and scalar issue queues (a single
    # dma_start costs ~700ns of queue issue time; one queue serializes the
    # wave). gpsimd stays DMA-free so tri/vext-ones/bv-broadcast run early.
    wqkv_s = consts.tile([128, KT, 6, 128], CD, tag="wqkv")
    xt_s = consts.tile([128, KT, T], CD, tag="xt")
    bqk_s = consts.tile([128, 4], F32, tag="bqk")
    bv_row = consts.tile([1, 256], F32, tag="bv_row")
    wp_s = consts.tile([128, 2, C], CD, tag="wp")
    nc.scalar.dma_start(bv_row[:], bv)
    nc.scalar.dma_start(bqk_s[:], bqk)
    for kt in range(KT):
        nc.sync.dma_start(
            xt_s[:, kt:kt + 1, 0:512], xt[:, kt:kt + 1, 0:512]
        )

    # fixed 128x128 causal triangle (keep where j <= c) for diagonal strips
    tri = consts.tile([128, 128], CD, tag="tri")
    nc.vector.memset(tri[:], 1.0)
    nc.gpsimd.affine_select(
        out=tri[:],
        in_=tri[:],
        compare_op=mybir.AluOpType.is_ge,
        fill=0.0,
        base=0,
        channel_multiplier=-1,
        pattern=[[1, 128]],
    )

    qk_t = [consts.tile([128, T], CD, tag=f"q{jt}", name=f"q{jt}")
            for jt in range(2)]
    # k for head pair p: rows 0-63 = head 2p feats, 64-127 = head 2p+1.
    # No zero-padding: S runs as two concurrent 64-row-tiled matmuls.
    kpair = [consts.tile([128, T], CD, tag=f"kp{p}", name=f"kp{p}")
             for p in range(2)]
    vext_s = consts.tile([128, TT, HPC * (HS + 1)], CD, tag="vext")
    vext4 = vext_s[:].rearrange("p t (h c) -> p t h c", c=HS + 1)
    nc.gpsimd.memset(vext4[:, :, :, HS], 1.0)  # ones columns
    yt_s = consts.tile([128, 2, T], CD, tag="yt")
    bv_bc = consts.tile([128, 256], F32, tag="bv_bc")
    nc.gpsimd.partition_broadcast(bv_bc[:], bv_row[:], channels=128)
    # wqkv per-kt pieces split over the scalar and gpsimd queues (gpsimd's
    # pieces are emitted after its tri/ones/broadcast ops so those stay
    # early); xt rest rides on sync behind the tb0 pieces
    for kt in range(KT):
        eng = nc.scalar if kt < 4 else nc.gpsimd
        eng.dma_start(wqkv_s[:, kt:kt + 1], wqkv[:, kt:kt + 1])
    # xt tb1 rides right behind the first wave in 2-kt pieces (phase-1 for
    # tb1 braids into ib0 at ~24us and stalled on late 512KB pieces); wp
    # is not needed until the first proj (~45us) so it goes last
    for tb in range(1, IB):
        tsl = slice(tb * 512, (tb + 1) * 512)
        eng = {1: nc.sync, 2: nc.scalar, 3: nc.sync}[tb]
        for kq in range(4):
            ks = slice(2 * kq, 2 * kq + 2)
            eng.dma_start(xt_s[:, ks, tsl], xt[:, ks, tsl])
        if tb == 1:
            nc.scalar.dma_start(wp_s[:, 0:1], wp[:, 0:1])
            nc.sync.dma_start(wp_s[:, 1:2], wp[:, 1:2])

    # pools: mm512 is shared by phase-1 QKV groups and the proj matmuls;
    # attn_sp is ONE 4-bank window holding S for a j-tile pair x both heads
    # so a single exp instruction (4-region strided AP) covers the window
    mm512 = ctx.enter_context(tc.tile_pool(name="mm512", bufs=2, space="PSUM"))
    attn_sp = ctx.enter_context(tc.tile_pool(name="attn_s", bufs=2, space="PSUM"))
    attn_yp = ctx.enter_context(tc.tile_pool(name="attn_y", bufs=2, space="PSUM"))
    pt_pool = ctx.enter_context(tc.tile_pool(name="pt", bufs=6))
    misc = ctx.enter_context(tc.tile_pool(name="misc", bufs=4))
    stage = ctx.enter_context(tc.tile_pool(name="stage", bufs=3))

    # warm-up matmuls: one accumulation group with a reader so Tile cannot
    # dead-code them (independent overwrites WERE eliminated: HAM stayed at
    # K=4/8 until ~13us). 8 cold MMs (~3.4us) bridge PE-busy until the
    # first DMA piece lands and flip HAM to 2.4GHz. They live in an attn_yp
    # bank (psy isn't needed until ~22us) so neither mm512 nor the phase-1
    # kt-outer groups wait on the warm_sink read.
    warm_ps = attn_yp.tile([128, 512], F32, tag="y", name="warm_ps")
    N_WARM = 16
    for i in range(N_WARM):
        nc.tensor.matmul(
            out=warm_ps[:], lhsT=dummy[:, 0:128], rhs=dummy[:],
            start=(i == 0), stop=(i == N_WARM - 1),
        )
    warm_sink = misc.tile([1, 8], F32, tag="warm_sink")
    nc.vector.tensor_copy(warm_sink[:], warm_ps[0:1, 0:8])
    # tiny exp up front pulls the ~2.7us ACT_TABLE_LOAD into the DMA wait
    act_warm = misc.tile([1, 16], CD, tag="actw")
    nc.scalar.activation(out=act_warm[:], in_=dummy[0:1, 0:16], func=Exp)

    def emit_ph1_block0():
        """Phase-1 for tb=0, kt-outer: all 8 output groups (4 qk + 4 v)
        stay open in PSUM and each arriving per-kt DMA piece (xt + wqkv)
        immediately unlocks 8 matmuls - compute starts with the first
        piece instead of waiting for half the wave."""
        qk_ps = [attn_sp.tile([128, 1024], F32, tag="s", name=f"p0qk{i}")
                 for i in range(2)]  # [jt2|jt3], [jt0|jt1]
        v_ps = [mm512.tile([128, 512], F32, tag="mm", name=f"p0v{i}")
                for i in range(2)]  # [tt0|tt1], [tt2|tt3]
        for kt in range(KT):
            for gi, jt in enumerate((2, 3, 0, 1)):
                nc.tensor.matmul(
                    out=qk_ps[gi // 2][:, (gi % 2) * 512:(gi % 2) * 512 + 512],
                    lhsT=wqkv_s[:, kt, jt, :],
                    rhs=xt_s[:, kt, 0:512],
                    start=(kt == 0),
                    stop=(kt == KT - 1),
                )
            for tt in range(4):
                # two 256-col v groups share one PSUM bank; start=True
                # clears has_written for the WHOLE bank, so only the
                # first group in the bank may issue it (the second's kt=0
                # lands on already-cleared bits and still accumulates)
                nc.tensor.matmul(
                    out=v_ps[tt // 2][:, (tt % 2) * 256:(tt % 2) * 256 + 256],
                    lhsT=xt_s[:, kt, tt * 128:(tt + 1) * 128],
                    rhs=wqkv_s[:, kt, 4:6, :].rearrange("p a b -> p (a b)"),
                    start=(kt == 0 and tt % 2 == 0),
                    stop=(kt == KT - 1),
                    skip_group_check=True,
                )
        for gi, jt in enumerate((2, 3, 0, 1)):
            ps = qk_ps[gi // 2][:, (gi % 2) * 512:(gi % 2) * 512 + 512]
            if jt < 2:
                nc.vector.tensor_scalar_add(
                    qk_t[jt][:, 0:512], ps, bqk_s[:, jt:jt + 1]
                )
            else:
                nc.vector.tensor_scalar_add(
                    kpair[jt - 2][:, 0:512], ps, bqk_s[:, jt:jt + 1]
                )
        for tt in range(4):
            nc.vector.tensor_tensor(
                out=vext4[:, tt, :, 0:HS],
                in0=v_ps[tt // 2][:, (tt % 2) * 256:(tt % 2) * 256 + 256]
                .rearrange("p (h c) -> p h c", c=HS),
                in1=bv_bc[:].rearrange("p (h c) -> p h c", c=HS),
                op=mybir.AluOpType.add,
            )

    def ph1_units(tb, which="kq"):
        """Phase-1 half-group units (4 matmuls each) that braid into
        attention's exp-paced windows. k groups first so the k chain
        overlaps the q groups. `which`: "kq" = full block, or "k"/"q"/"v"
        subsets (used to defer tb=3 work into the last attention block)."""
        tsl = slice(tb * 512, (tb + 1) * 512)
        units = []

        def qk_unit(jt, half, cell):
            def emit():
                if half == 0:
                    cell["ps"] = mm512.tile(
                        [128, 512], F32, tag="mm", name=f"qk{tb}_{jt}"
                    )
                ps = cell["ps"]
                for kt in range(4 * half, 4 * half + 4):
                    nc.tensor.matmul(
                        out=ps[:],
                        lhsT=wqkv_s[:, kt, jt, :],
                        rhs=xt_s[:, kt, tsl],
                        start=(kt == 0),
                        stop=(kt == KT - 1),
                    )
                if half == 0:
                    return
                if jt < 2:  # q
                    nc.vector.tensor_scalar_add(
                        qk_t[jt][:, tsl], ps[:], bqk_s[:, jt:jt + 1]
                    )
                else:  # k pair tile: one add covers both heads
                    nc.vector.tensor_scalar_add(
                        kpair[jt - 2][:, tsl], ps[:], bqk_s[:, jt:jt + 1]
                    )
            return emit

        def v_unit(tt, half, cell):
            def emit():
                if half == 0:
                    cell["ps"] = mm512.tile(
                        [128, 512], F32, tag="mm", name=f"v{tt}"
                    )
                ps = cell["ps"]
                for kt in range(4 * half, 4 * half + 4):
                    nc.tensor.matmul(
                        out=ps[:, 0:256],
                        lhsT=xt_s[:, kt, tt * 128:(tt + 1) * 128],
                        rhs=wqkv_s[:, kt, 4:6, :].rearrange("p a b -> p (a b)"),
                        start=(kt == 0),
                        stop=(kt == KT - 1),
                    )
                if half == 1:
                    nc.vector.tensor_tensor(
                        out=vext4[:, tt, :, 0:HS],
                        in0=ps[:, 0:256].rearrange("p (h c) -> p h c", c=HS),
                        in1=bv_bc[:].rearrange("p (h c) -> p h c", c=HS),
                        op=mybir.AluOpType.add,
                    )
            return emit

        jts = {"k": (2, 3), "q": (0, 1), "kq": (2, 3, 0, 1),
               "all": (2, 3, 0, 1), "v": ()}[which]
        for jt in jts:
            cell = {}
            units += [qk_unit(jt, 0, cell), qk_unit(jt, 1, cell)]
        if which in ("all", "v"):
            for tt in range(tb * 4, tb * 4 + 4):
                cell = {}
                units += [v_unit(tt, 0, cell), v_unit(tt, 1, cell)]
        return units

    def proj_units(ib):
        units = []

        def unit(tloc):
            def emit():
                emit_proj_tloc(ib, tloc)
            return emit

        for tloc in range(4):
            units.append(unit(tloc))
        return units

    def emit_proj_tloc(ib, tloc):
        ttp = ib * 4 + tloc
        st = stage.tile([128, C], CD, tag="st", name=f"st{ttp}")
        for eb in range(2):
            psp = mm512.tile([128, 512], F32, tag="mm", name=f"pj{ttp}_{eb}")
            for dt in range(2):
                nc.tensor.matmul(
                    out=psp[:],
                    lhsT=yt_s[:, dt, ttp * 128:(ttp + 1) * 128],
                    rhs=wp_s[:, dt, eb * 512:(eb + 1) * 512],
                    start=(dt == 0),
                    stop=(dt == 1),
                )
            nc.vector.tensor_copy(st[:, eb * 512:(eb + 1) * 512], psp[:])
        nc.sync.dma_start(out[ttp * 128:(ttp + 1) * 128, :], st[:])

    def emit_proj_tloc_tail(ttp):
        """Tail projection t-tile: eb0 psum from mm512, eb1 from attn_yp
        (its banks are free once the last yt-mul consumed psy) so two
        t-tiles pipeline; casts split scalar/vector; DMA from the idle
        sync queue (issuing from scalar would delay the next cast)."""
        st = stage.tile([128, C], CD, tag="st", name=f"st{ttp}")
        for eb in range(2):
            pool = mm512 if eb == 0 else attn_sp
            psp = pool.tile(
                [128, 512], F32,
                tag="mm" if eb == 0 else "s", name=f"pj{ttp}_{eb}",
            )
            for dt in range(2):
                nc.tensor.matmul(
                    out=psp[:],
                    lhsT=yt_s[:, dt, ttp * 128:(ttp + 1) * 128],
                    rhs=wp_s[:, dt, eb * 512:(eb + 1) * 512],
                    start=(dt == 0),
                    stop=(dt == 1),
                )
            if eb == 0:
                nc.scalar.copy(st[:, 0:512], psp[:])
            else:
                nc.vector.tensor_copy(st[:, 512:1024], psp[:])
            nc.sync.dma_start(
                out[ttp * 128:(ttp + 1) * 128, eb * 512:(eb + 1) * 512],
                st[:, eb * 512:(eb + 1) * 512],
            )

    def emit_attn_block(ib, fill):
        isl = slice(ib * 512, (ib + 1) * 512)
        njt = 4 * ib + 4
        for p in range(2):  # head pairs (2p, 2p+1)
            qt = qk_t[p]
            last_pair = (ib == IB - 1 and p == 1)
            if last_pair:
                # i-column-split psy: bank X holds both heads' cols 0:256
                # (A at 0:256, B at 256:512), bank Y cols 256:512. Lets the
                # 0:256 normalize + proj of t-tiles 12/13 overlap the final
                # PV matmuls (which only touch bank Y) - the kernel tail.
                psyX = attn_yp.tile([HS + 1, 512], F32, tag="y", name="yX")
                psyY = attn_yp.tile([HS + 1, 512], F32, tag="y", name="yY")
            else:
                psy = [attn_yp.tile([HS + 1, 512], F32, tag="y",
                                    name=f"y{ib}_{p}_{u}") for u in range(2)]

            def emit_pv(items, last):
                for (pt, j, cols, off) in items:
                    w = 512 - off
                    for half in range(2):
                        if not last_pair:
                            nc.tensor.matmul(
                                out=psy[half][:, off:512],
                                lhsT=vext4[:, j, 2 * p + half, :],
                                rhs=pt[:, cols[half]:cols[half] + w],
                                start=(j == 0),
                                stop=(last and j == njt - 1),
                            )
                            continue
                        s = half * 256
                        if off < 256:  # piece into bank X (i-cols off:256)
                            nc.tensor.matmul(
                                out=psyX[:, s + off:s + 256],
                                lhsT=vext4[:, j, 2 * p + half, :],
                                rhs=pt[:, cols[half]:cols[half] + 256 - off],
                                start=(j == 0 and half == 0),
                                stop=(j == njt - 3 and half == 1),
                                skip_group_check=True,
                            )
                        roff = max(off, 256)  # bank Y (i-cols roff:512)
                        nc.tensor.matmul(
                            out=psyY[:, s + roff - 256:s + 256],
                            lhsT=vext4[:, j, 2 * p + half, :],
                            rhs=pt[:, cols[half] + roff - off:cols[half] + w],
                            start=(j == 0 and half == 0),
                            stop=(last and j == njt - 1 and half == 1),
                            skip_group_check=True,
                        )

            prev = []
            for g in range(njt // 2):
                fill()
                cur = []
                # both j-tiles' S pairs first (one 64-row-mode burst on the
                # PE), then the exps/masks chase on scalar/vector. The two
                # heads' concurrent row-tiled matmuls hit different banks
                # (A at col 0, B at col 512).
                for j in (2 * g, 2 * g + 1):
                    psS = attn_sp.tile([128, 1024], F32, tag="s")
                    pt = pt_pool.tile([128, 1024], CD, tag="pt")
                    off = max(0, 128 * j - 512 * ib)
                    w = 512 - off
                    cols = [0, 512]
                    for half in range(2):
                        nc.tensor.matmul(
                            out=psS[:, cols[half]:cols[half] + w],
                            lhsT=kpair[p][64 * half:64 * half + 64,
                                          j * 128:(j + 1) * 128],
                            rhs=qt[64 * half:64 * half + 64,
                                   ib * 512 + off:(ib + 1) * 512],
                            start=True,
                            stop=True,
                        )
                    cur.append((psS, pt, j, cols, off))
                done = []
                for (psS, pt, j, cols, off) in cur:
                    w = 512 - off
                    # one exp over both banks via a strided 2-region AP
                    # ([128, 2, w]) - no wasted columns on diagonal tiles
                    nc.scalar.activation(
                        out=pt[:].rearrange("p (b c) -> p b c", b=2)[:, :, 0:w],
                        in_=psS[:].rearrange("p (b c) -> p b c", b=2)[:, :, 0:w],
                        func=Exp,
                        scale=0.125,
                    )
                    if 128 * j >= 512 * ib:  # diagonal tile -> mask both
                        for c in cols:
                            nc.vector.tensor_mul(
                                pt[:, c:c + 128], pt[:, c:c + 128], tri[:],
                            )
                    done.append((pt, j, cols, off))
                if prev:
                    emit_pv(prev, last=False)
                prev = done
            emit_pv(prev, last=True)
            if not last_pair:
                lrows, lbcs = [], []
                for half in range(2):
                    lrow = misc.tile([1, 512], F32, tag="lrow")
                    nc.vector.tensor_copy(lrow[:], psy[half][HS:HS + 1, :])
                    lrows.append(lrow)
                for half in range(2):
                    linv = misc.tile([1, 512], F32, tag="linv")
                    nc.vector.reciprocal_approx_fast(linv[:], lrows[half][:])
                    lbc = misc.tile([64, 512], F32, tag="lbc")
                    nc.gpsimd.partition_broadcast(
                        lbc[:], linv[:], channels=64
                    )
                    lbcs.append(lbc)
                for half in range(2):
                    nc.vector.tensor_mul(
                        yt_s[half * 64:half * 64 + 64, p, isl],
                        psy[half][0:HS, :], lbcs[half][:]
                    )
                continue

            # last pair: bank-X normalize overlaps the bank-Y final PVs;
            # proj t-tiles 12/13 (X columns) overlap the bank-Y normalize.
            # One recip+broadcast covers both heads (their l-rows packed
            # side by side), halving the serial chain.
            def norm_bank(bank, yt_c0):
                lrow = misc.tile([1, 512], F32, tag="lrow")
                nc.scalar.copy(lrow[:, 0:256], bank[HS:HS + 1, 0:256])
                nc.vector.tensor_copy(lrow[:, 256:512],
                                      bank[HS:HS + 1, 256:512])
                linv = misc.tile([1, 512], F32, tag="linv")
                nc.vector.reciprocal_approx_fast(linv[:], lrow[:])
                lbc = misc.tile([64, 512], F32, tag="lbc")
                nc.gpsimd.partition_broadcast(lbc[:], linv[:], channels=64)
                for half in range(2):
                    s = half * 256
                    nc.vector.tensor_mul(
                        yt_s[half * 64:half * 64 + 64, p,
                             yt_c0:yt_c0 + 256],
                        bank[0:HS, s:s + 256], lbc[:, s:s + 256]
                    )

            base = ib * 512
            norm_bank(psyX, base)
            norm_bank(psyY, base + 256)
            for tloc in range(4):
                emit_proj_tloc_tail(ib * 4 + tloc)

    # braid plan: ib=3 is ACT-bound so it gets tb3's k/v units (not needed
    # until its j>=12 tiles) plus proj(1) and proj(2). fills_per_window=2
    # in ib0 (few windows) and ib3 (emission of k units must stay ahead of
    # the j=12 S matmul in window 7 of head-pair 0).
    fq = []
    fill_sched = [[1]]
    fill_idx = [0]

    def fill():
        sched = fill_sched[0]
        n = sched[min(fill_idx[0], len(sched) - 1)]
        fill_idx[0] += 1
        for _ in range(n):
            if fq:
                fq.pop(0)()

    emit_ph1_block0()
    plan = {
        0: ph1_units(1, "all"),
        1: ph1_units(2, "kq") + proj_units(0),
        2: ph1_units(2, "v") + ph1_units(3, "q") + proj_units(1),
        3: ph1_units(3, "k") + ph1_units(3, "v") + proj_units(2),
    }
    # pops per window, front-loaded to meet emission deadlines (k/v units
    # before the S/PV that consume them) but spread so the ACT-bound late
    # ib3 windows keep the PE fed (HAM re-throttles after ~3.4us of low
    # PE activity, halving the clock for everything that follows)
    fills = {
        0: [2],
        1: [1, 1, 1, 1, 2, 2, 2, 2],
        2: [1, 1, 1, 1, 2, 2, 2, 2, 2, 1, 1, 1],
        3: [2, 2, 2, 2, 1, 1, 1, 1, 1, 1, 1, 1, 1, 0, 0, 0],
    }
    for ib in range(IB):
        fq.extend(plan[ib])
        fill_sched[0] = fills[ib]
        fill_idx[0] = 0
        emit_attn_block(ib, fill)
        while fq:
            fq.pop(0)()


def build():
    if "nc" in _CACHE:
        return _CACHE["nc"]
    nc = bacc.Bacc(
        "TRN2", target_bir_lowering=False, debug=False, num_devices=N_CORES
    )
    aps = {
        "xt": nc.dram_tensor("xt", [128, KT, T], CD, kind="ExternalInput").ap(),
        "wqkv": nc.dram_tensor("wqkv", [128, KT, 6, 128], CD, kind="ExternalInput").ap(),
        "bqk": nc.dram_tensor("bqk", [128, 4], F32, kind="ExternalInput").ap(),
        "bv": nc.dram_tensor("bv", [1, 256], F32, kind="ExternalInput").ap(),
        "wp": nc.dram_tensor("wp", [128, 2, C], CD, kind="ExternalInput").ap(),
        "out": nc.dram_tensor("out", [T, C], CD, kind="ExternalOutput").ap(),
    }
    from contextlib import ExitStack

    with tile.TileContext(nc) as tc:
        with ExitStack() as ctx:
            _emit(nc, tc, ctx, aps)
    nc.compile()
    _CACHE["nc"] = nc
    return nc


def make_in_maps(x, W_attn, b_attn, W_proj, b_proj):
    x = np.asarray(x, dtype=np.float32)
    W_attn = np.asarray(W_attn, dtype=np.float32)
    b_attn = np.asarray(b_attn, dtype=np.float32)
    W_proj = np.asarray(W_proj, dtype=np.float32)

    in_maps = []
    xt_b = {}
    for b in range(B):
        xt = np.ascontiguousarray(x[b].T)  # [C, T]
        xt_b[b] = (
            xt.reshape(KT, 128, T).transpose(1, 0, 2).astype(CD_NP)
        )
    for core in range(N_CORES):
        b = core // 4
        g = core % 4
        fs = slice(256 * g, 256 * g + 256)  # feature cols for this head group
        wq = W_attn[:, fs]
        wk = W_attn[:, C + 256 * g: C + 256 * g + 256]
        wv = W_attn[:, 2 * C + 256 * g: 2 * C + 256 * g + 256]
        wqkv = np.concatenate([wq, wk, wv], axis=1)  # [1024, 768]
        bq = b_attn[fs]
        bk = b_attn[C + 256 * g: C + 256 * g + 256]
        bv = b_attn[2 * C + 256 * g: 2 * C + 256 * g + 256]
        in_maps.append({
            "xt": xt_b[b],
            "wqkv": np.ascontiguousarray(
                wqkv.reshape(KT, 128, 6, 128).transpose(1, 0, 2, 3)
            ).astype(CD_NP),
            "bqk": np.ascontiguousarray(
                np.concatenate([bq, bk]).reshape(4, 128).T
            ).astype(np.float32),
            "bv": bv[None, :].astype(np.float32),
            "wp": np.ascontiguousarray(
                W_proj[fs, :].reshape(2, 128, C).transpose(1, 0, 2)
            ).astype(CD_NP),
        })
    return in_maps


def _ensure_ntff_hook():
    """Recreate the missing antenv.axon_hooks NTFF-profile shim (see
    trn_agent_boot/trn_boot.py) so run_bass_kernel_spmd(trace=True) works."""
    import contextlib
    import ctypes
    import types

    try:
        from antenv.axon_hooks import get_axon_ntff_profile_hook  # noqa: F401

        return
    except ImportError:
        pass

    mod = types.ModuleType("antenv.axon_hooks")
    _holder = {"hook": None}
    mod.set_axon_ntff_profile_hook = lambda h: _holder.__setitem__("hook", h)
    mod.get_axon_ntff_profile_hook = lambda: _holder["hook"]
    sys.modules["antenv.axon_hooks"] = mod
    import antenv

    antenv.axon_hooks = mod

    so_path = "/opt/axon/libaxon_pjrt.so"
    if not os.path.exists(so_path):
        return
    lib = ctypes.CDLL(so_path)
    if not hasattr(lib, "axon_start_nrt_profile"):
        return
    lib.axon_start_nrt_profile.argtypes = [
        ctypes.POINTER(ctypes.c_int64),
        ctypes.c_size_t,
    ]
    lib.axon_start_nrt_profile.restype = ctypes.c_int64
    lib.axon_stop_nrt_profile.argtypes = [ctypes.c_char_p]
    lib.axon_stop_nrt_profile.restype = ctypes.c_int64

    @contextlib.contextmanager
    def _hook(output_dir, device_ids):
        import jax

        jax.devices()
        if device_ids:
            ids = (ctypes.c_int64 * len(device_ids))(*device_ids)
            rc = lib.axon_start_nrt_profile(ids, len(device_ids))
        else:
            rc = lib.axon_start_nrt_profile(None, 0)
        if rc != 0:
            raise RuntimeError(f"axon_start_nrt_profile rc={rc}")
        try:
            yield
        finally:
            n = lib.axon_stop_nrt_profile(str(output_dir).encode())
            if n <= 0:
                print(f"ntff profile: rc={n}, nothing written to {output_dir}")

    mod.set_axon_ntff_profile_hook(_hook)


def kernel(x, W_attn, b_attn, W_proj, b_proj):
    global LAST_RESULT
    nc = build()
    in_maps = make_in_maps(x, W_attn, b_attn, W_proj, b_proj)
    b_proj = np.asarray(b_proj, dtype=np.float32)
    trace = os.environ.get("KERNEL_TRACE", "0") == "1"
    if trace:
        _ensure_ntff_hook()
        import concourse.bass_utils as _bu

        _bu.upload_artifacts = lambda tmpdir: f"local://{tmpdir}"
    res = run_bass_kernel_spmd(
        nc, in_maps, core_ids=list(range(N_CORES)), trace=trace
    )
    LAST_RESULT = res
    outs = [res.results[i]["out"] for i in range(N_CORES)]
    y = np.empty((B, T, C), dtype=np.float32)
    for b in range(B):
        acc = outs[4 * b].astype(np.float32)
        for g in range(1, 4):
            acc = acc + outs[4 * b + g].astype(np.float32)
        y[b] = acc + b_proj
    return y


# revision 50
# speedup vs baseline: 1.0233x; 1.0233x over previous
"""Causal self-attention (B=2, T=2048, C=1024, nh=16) on 8 TRN2 NeuronCores.

Sharding: core c -> batch b = c//4, head group g = c%4 (4 heads each).
Each core computes QKV projections for its heads, causal attention, and a
partial output projection (W_proj rows for its heads). The four partials per
batch are summed on the host, which also adds b_proj.

Layouts (per core, hardcoded):
  xt    [128, 8, 2048]     x[b].T tiles:  xt[p, kt, t] = x[b, t, kt*128+p]
  wqkv  [128, 8, 6, 128]   W_attn q|k|v cols for this core's heads
  bqk   [128, 4] f32       b_attn q|k (per-partition bias)
  bv    [1, 256] f32       b_attn v (broadcast on device)
  wp    [128, 2, 1024]     W_proj rows for this core's heads
  out   [2048, 1024] bf16  partial (x[b] @ ... for this head group)

In-kernel dataflow (all matmuls bf16 with fp32 PSUM accumulation):
  qT,kT = (W.T @ x.T)      [feat, t] layout  (lhsT=W tile, rhs=xT)
  v     = (x @ Wv)         [t, feat] layout, written into vext (65-col per
                           head, ones column -> l rides along in PV)
  S^T   = k @ q.T          [j, i] layout. The contraction is only hs=64, so
          the two heads of a pair run CONCURRENTLY as 64-row-tiled matmuls
          (tile_position (0,0) and (64,0)): head A's k/q feats live on
          partitions 0-63, head B's on 64-127. One j-tile costs ~N cycles
          for BOTH heads instead of 2N with the old zero-padded K=128 form.
  P^T   = exp(S^T/8), masked on diagonal tiles (mult by 0/1 triangle)
  y^T,l = [v|1].T @ P^T    [d, i] layout, row 64 = l = sum_j P  (128x128)
  yT    = y^T * (1/l broadcast)
  out   = yT.T @ Wp        (bias added on host)

Schedule: phase-1 for tb=0 runs up front; attention i-block ib then runs
with deferred work braided into its exp-paced windows. ib=3 is ACT-bound
(~34us of exp vs ~25us of its own S+PV), so tb=3's k/v projections and the
proj blocks of ib=1,2 are deferred into it.
"""

import os
import sys

sys.path.insert(0, "/opt/trn_rl_repo")
os.environ.setdefault("MYCRO_LOCAL_CACHE", "1")

import ml_dtypes
import numpy as np

import concourse.bass as bass
import concourse.mybir as mybir
import concourse.tile as tile
from concourse import bacc
from concourse.bass_utils import run_bass_kernel_spmd

B, T, C, NH, HS = 2, 2048, 1024, 16, 64
HPC = 4  # heads per core
N_CORES = 8
KT = C // 128  # 8 contraction tiles over C
TT = T // 128  # 16 tiles over T
IB = T // 512  # 4 i-blocks over T
F32 = mybir.dt.float32

CD = mybir.dt.bfloat16
CD_NP = ml_dtypes.bfloat16

LAST_RESULT = None
_CACHE = {}


def _emit(nc, tc, ctx, aps):
    xt, wqkv, bqk, bv, wp, out = (
        aps["xt"], aps["wqkv"], aps["bqk"], aps["bv"], aps["wp"], aps["out"],
    )
    Exp = mybir.ActivationFunctionType.Exp

    consts = ctx.enter_context(tc.tile_pool(name="consts", bufs=1))

    dummy = consts.tile([128, 512], CD, tag="dummy")
    nc.vector.memset(dummy[:], 1.0)

    # --- persistent SBUF tensors. First-wave DMAs (x tb=0 + weights) are
    # 2-kt pieces split between the sync and scalar issue queues (a single
    # dma_start costs ~700ns of queue issue time; one queue serializes the
    # wave). gpsimd stays DMA-free so tri/vext-ones/bv-broadcast run early.
    wqkv_s = consts.tile([128, KT, 6, 128], CD, tag="wqkv")
    xt_s = consts.tile([128, KT, T], CD, tag="xt")
    bqk_s = consts.tile([128, 4], F32, tag="bqk")
    bv_row = consts.tile([1, 256], F32, tag="bv_row")
    wp_s = consts.tile([128, 2, C], CD, tag="wp")
    nc.scalar.dma_start(bv_row[:], bv)
    nc.scalar.dma_start(bqk_s[:], bqk)
    for kt in range(KT):
        nc.sync.dma_start(
            xt_s[:, kt:kt + 1, 0:512], xt[:, kt:kt + 1, 0:512]
        )

    # fixed 128x128 causal triangle (keep where j <= c) for diagonal strips
    tri = consts.tile([128, 128], CD, tag="tri")
    nc.vector.memset(tri[:], 1.0)
    nc.gpsimd.affine_select(
        out=tri[:],
        in_=tri[:],
        compare_op=mybir.AluOpType.is_ge,
        fill=0.0,
        base=0,
        channel_multiplier=-1,
        pattern=[[1, 128]],
    )

    qk_t = [consts.tile([128, T], CD, tag=f"q{jt}", name=f"q{jt}")
            for jt in range(2)]
    # k for head pair p: rows 0-63 = head 2p feats, 64-127 = head 2p+1.
    # No zero-padding: S runs as two concurrent 64-row-tiled matmuls.
    kpair = [consts.tile([128, T], CD, tag=f"kp{p}", name=f"kp{p}")
             for p in range(2)]
    vext_s = consts.tile([128, TT, HPC * (HS + 1)], CD, tag="vext")
    vext4 = vext_s[:].rearrange("p t (h c) -> p t h c", c=HS + 1)
    nc.gpsimd.memset(vext4[:, :, :, HS], 1.0)  # ones columns
    yt_s = consts.tile([128, 2, T], CD, tag="yt")
    bv_bc = consts.tile([128, 256], F32, tag="bv_bc")
    nc.gpsimd.partition_broadcast(bv_bc[:], bv_row[:], channels=128)
    # wqkv per-kt pieces split over the scalar and gpsimd queues (gpsimd's
    # pieces are emitted after its tri/ones/broadcast ops so those stay
    # early); xt rest rides on sync behind the tb0 pieces
    for kt in range(KT):
        eng = nc.scalar if kt < 4 else nc.gpsimd
        eng.dma_start(wqkv_s[:, kt:kt + 1], wqkv[:, kt:kt + 1])
    nc.sync.dma_start(wp_s[:], wp)
    for tb in range(1, IB):
        tsl = slice(tb * 512, (tb + 1) * 512)
        for hf in range(2):
            ks = slice(4 * hf, 4 * hf + 4)
            nc.sync.dma_start(xt_s[:, ks, tsl], xt[:, ks, tsl])

    # pools: mm512 is shared by phase-1 QKV groups and the proj matmuls;
    # attn_sp is ONE 4-bank window holding S for a j-tile pair x both heads
    # so a single exp instruction (4-region strided AP) covers the window
    mm512 = ctx.enter_context(tc.tile_pool(name="mm512", bufs=2, space="PSUM"))
    attn_sp = ctx.enter_context(tc.tile_pool(name="attn_s", bufs=2, space="PSUM"))
    attn_yp = ctx.enter_context(tc.tile_pool(name="attn_y", bufs=2, space="PSUM"))
    pt_pool = ctx.enter_context(tc.tile_pool(name="pt", bufs=6))
    misc = ctx.enter_context(tc.tile_pool(name="misc", bufs=4))
    stage = ctx.enter_context(tc.tile_pool(name="stage", bufs=3))

    # warm-up matmuls: one accumulation group with a reader so Tile cannot
    # dead-code them (independent overwrites WERE eliminated: HAM stayed at
    # K=4/8 until ~13us). 8 cold MMs (~3.4us) bridge PE-busy until the
    # first DMA piece lands and flip HAM to 2.4GHz. They live in an attn_yp
    # bank (psy isn't needed until ~22us) so neither mm512 nor the phase-1
    # kt-outer groups wait on the warm_sink read.
    warm_ps = attn_yp.tile([128, 512], F32, tag="y", name="warm_ps")
    N_WARM = 16
    for i in range(N_WARM):
        nc.tensor.matmul(
            out=warm_ps[:], lhsT=dummy[:, 0:128], rhs=dummy[:],
            start=(i == 0), stop=(i == N_WARM - 1),
        )
    warm_sink = misc.tile([1, 8], F32, tag="warm_sink")
    nc.vector.tensor_copy(warm_sink[:], warm_ps[0:1, 0:8])
    # tiny exp up front pulls the ~2.7us ACT_TABLE_LOAD into the DMA wait
    act_warm = misc.tile([1, 16], CD, tag="actw")
    nc.scalar.activation(out=act_warm[:], in_=dummy[0:1, 0:16], func=Exp)

    def emit_ph1_block0():
        """Phase-1 for tb=0, kt-outer: all 8 output groups (4 qk + 4 v)
        stay open in PSUM and each arriving per-kt DMA piece (xt + wqkv)
        immediately unlocks 8 matmuls - compute starts with the first
        piece instead of waiting for half the wave."""
        qk_ps = [attn_sp.tile([128, 1024], F32, tag="s", name=f"p0qk{i}")
                 for i in range(2)]  # [jt2|jt3], [jt0|jt1]
        v_ps = [mm512.tile([128, 512], F32, tag="mm", name=f"p0v{i}")
                for i in range(2)]  # [tt0|tt1], [tt2|tt3]
        for kt in range(KT):
            for gi, jt in enumerate((2, 3, 0, 1)):
                nc.tensor.matmul(
                    out=qk_ps[gi // 2][:, (gi % 2) * 512:(gi % 2) * 512 + 512],
                    lhsT=wqkv_s[:, kt, jt, :],
                    rhs=xt_s[:, kt, 0:512],
                    start=(kt == 0),
                    stop=(kt == KT - 1),
                )
            for tt in range(4):
                # two 256-col v groups share one PSUM bank; start=True
                # clears has_written for the WHOLE bank, so only the
                # first group in the bank may issue it (the second's kt=0
                # lands on already-cleared bits and still accumulates)
                nc.tensor.matmul(
                    out=v_ps[tt // 2][:, (tt % 2) * 256:(tt % 2) * 256 + 256],
                    lhsT=xt_s[:, kt, tt * 128:(tt + 1) * 128],
                    rhs=wqkv_s[:, kt, 4:6, :].rearrange("p a b -> p (a b)"),
                    start=(kt == 0 and tt % 2 == 0),
                    stop=(kt == KT - 1),
                    skip_group_check=True,
                )
        for gi, jt in enumerate((2, 3, 0, 1)):
            ps = qk_ps[gi // 2][:, (gi % 2) * 512:(gi % 2) * 512 + 512]
            if jt < 2:
                nc.vector.tensor_scalar_add(
                    qk_t[jt][:, 0:512], ps, bqk_s[:, jt:jt + 1]
                )
            else:
                nc.vector.tensor_scalar_add(
                    kpair[jt - 2][:, 0:512], ps, bqk_s[:, jt:jt + 1]
                )
        for tt in range(4):
            nc.vector.tensor_tensor(
                out=vext4[:, tt, :, 0:HS],
                in0=v_ps[tt // 2][:, (tt % 2) * 256:(tt % 2) * 256 + 256]
                .rearrange("p (h c) -> p h c", c=HS),
                in1=bv_bc[:].rearrange("p (h c) -> p h c", c=HS),
                op=mybir.AluOpType.add,
            )

    def ph1_units(tb, which="kq"):
        """Phase-1 half-group units (4 matmuls each) that braid into
        attention's exp-paced windows. k groups first so the k chain
        overlaps the q groups. `which`: "kq" = full block, or "k"/"q"/"v"
        subsets (used to defer tb=3 work into the last attention block)."""
        tsl = slice(tb * 512, (tb + 1) * 512)
        units = []

        def qk_unit(jt, half, cell):
            def emit():
                if half == 0:
                    cell["ps"] = mm512.tile(
                        [128, 512], F32, tag="mm", name=f"qk{tb}_{jt}"
                    )
                ps = cell["ps"]
                for kt in range(4 * half, 4 * half + 4):
                    nc.tensor.matmul(
                        out=ps[:],
                        lhsT=wqkv_s[:, kt, jt, :],
                        rhs=xt_s[:, kt, tsl],
                        start=(kt == 0),
                        stop=(kt == KT - 1),
                    )
                if half == 0:
                    return
                if jt < 2:  # q
                    nc.vector.tensor_scalar_add(
                        qk_t[jt][:, tsl], ps[:], bqk_s[:, jt:jt + 1]
                    )
                else:  # k pair tile: one add covers both heads
                    nc.vector.tensor_scalar_add(
                        kpair[jt - 2][:, tsl], ps[:], bqk_s[:, jt:jt + 1]
                    )
            return emit

        def v_unit(tt, half, cell):
            def emit():
                if half == 0:
                    cell["ps"] = mm512.tile(
                        [128, 512], F32, tag="mm", name=f"v{tt}"
                    )
                ps = cell["ps"]
                for kt in range(4 * half, 4 * half + 4):
                    nc.tensor.matmul(
                        out=ps[:, 0:256],
                        lhsT=xt_s[:, kt, tt * 128:(tt + 1) * 128],
                        rhs=wqkv_s[:, kt, 4:6, :].rearrange("p a b -> p (a b)"),
                        start=(kt == 0),
                        stop=(kt == KT - 1),
                    )
                if half == 1:
                    nc.vector.tensor_tensor(
                        out=vext4[:, tt, :, 0:HS],
                        in0=ps[:, 0:256].rearrange("p (h c) -> p h c", c=HS),
                        in1=bv_bc[:].rearrange("p (h c) -> p h c", c=HS),
                        op=mybir.AluOpType.add,
                    )
            return emit

        jts = {"k": (2, 3), "q": (0, 1), "kq": (2, 3, 0, 1),
               "all": (2, 3, 0, 1), "v": ()}[which]
        for jt in jts:
            cell = {}
            units += [qk_unit(jt, 0, cell), qk_unit(jt, 1, cell)]
        if which in ("all", "v"):
            for tt in range(tb * 4, tb * 4 + 4):
                cell = {}
                units += [v_unit(tt, 0, cell), v_unit(tt, 1, cell)]
        return units

    def proj_units(ib):
        units = []

        def unit(tloc):
            def emit():
                emit_proj_tloc(ib, tloc)
            return emit

        for tloc in range(4):
            units.append(unit(tloc))
        return units

    def emit_proj_tloc(ib, tloc):
        ttp = ib * 4 + tloc
        st = stage.tile([128, C], CD, tag="st", name=f"st{ttp}")
        for eb in range(2):
            psp = mm512.tile([128, 512], F32, tag="mm", name=f"pj{ttp}_{eb}")
            for dt in range(2):
                nc.tensor.matmul(
                    out=psp[:],
                    lhsT=yt_s[:, dt, ttp * 128:(ttp + 1) * 128],
                    rhs=wp_s[:, dt, eb * 512:(eb + 1) * 512],
                    start=(dt == 0),
                    stop=(dt == 1),
                )
            nc.vector.tensor_copy(st[:, eb * 512:(eb + 1) * 512], psp[:])
        nc.sync.dma_start(out[ttp * 128:(ttp + 1) * 128, :], st[:])

    def emit_proj_tloc_tail(ttp):
        """Tail projection t-tile: eb0 psum from mm512, eb1 from attn_yp
        (its banks are free once the last yt-mul consumed psy) so two
        t-tiles pipeline; casts split scalar/vector; DMA from the idle
        sync queue (issuing from scalar would delay the next cast)."""
        st = stage.tile([128, C], CD, tag="st", name=f"st{ttp}")
        for eb in range(2):
            pool = mm512 if eb == 0 else attn_sp
            psp = pool.tile(
                [128, 512], F32,
                tag="mm" if eb == 0 else "s", name=f"pj{ttp}_{eb}",
            )
            for dt in range(2):
                nc.tensor.matmul(
                    out=psp[:],
                    lhsT=yt_s[:, dt, ttp * 128:(ttp + 1) * 128],
                    rhs=wp_s[:, dt, eb * 512:(eb + 1) * 512],
                    start=(dt == 0),
                    stop=(dt == 1),
                )
            if eb == 0:
                nc.scalar.copy(st[:, 0:512], psp[:])
            else:
                nc.vector.tensor_copy(st[:, 512:1024], psp[:])
            nc.sync.dma_start(
                out[ttp * 128:(ttp + 1) * 128, eb * 512:(eb + 1) * 512],
                st[:, eb * 512:(eb + 1) * 512],
            )

    def emit_attn_block(ib, fill):
        isl = slice(ib * 512, (ib + 1) * 512)
        njt = 4 * ib + 4
        for p in range(2):  # head pairs (2p, 2p+1)
            qt = qk_t[p]
            last_pair = (ib == IB - 1 and p == 1)
            if last_pair:
                # i-column-split psy: bank X holds both heads' cols 0:256
                # (A at 0:256, B at 256:512), bank Y cols 256:512. Lets the
                # 0:256 normalize + proj of t-tiles 12/13 overlap the final
                # PV matmuls (which only touch bank Y) - the kernel tail.
                psyX = attn_yp.tile([HS + 1, 512], F32, tag="y", name="yX")
                psyY = attn_yp.tile([HS + 1, 512], F32, tag="y", name="yY")
            else:
                psy = [attn_yp.tile([HS + 1, 512], F32, tag="y",
                                    name=f"y{ib}_{p}_{u}") for u in range(2)]

            def emit_pv(items, last):
                for (pt, j, cols, off) in items:
                    w = 512 - off
                    for half in range(2):
                        if not last_pair:
                            nc.tensor.matmul(
                                out=psy[half][:, off:512],
                                lhsT=vext4[:, j, 2 * p + half, :],
                                rhs=pt[:, cols[half]:cols[half] + w],
                                start=(j == 0),
                                stop=(last and j == njt - 1),
                            )
                            continue
                        s = half * 256
                        if off < 256:  # piece into bank X (i-cols off:256)
                            nc.tensor.matmul(
                                out=psyX[:, s + off:s + 256],
                                lhsT=vext4[:, j, 2 * p + half, :],
                                rhs=pt[:, cols[half]:cols[half] + 256 - off],
                                start=(j == 0 and half == 0),
                                stop=(j == njt - 3 and half == 1),
                                skip_group_check=True,
                            )
                        roff = max(off, 256)  # bank Y (i-cols roff:512)
                        nc.tensor.matmul(
                            out=psyY[:, s + roff - 256:s + 256],
                            lhsT=vext4[:, j, 2 * p + half, :],
                            rhs=pt[:, cols[half] + roff - off:cols[half] + w],
                            start=(j == 0 and half == 0),
                            stop=(last and j == njt - 1 and half == 1),
                            skip_group_check=True,
                        )

            prev = []
            for g in range(njt // 2):
                fill()
                cur = []
                # both j-tiles' S pairs first (one 64-row-mode burst on the
                # PE), then the exps/masks chase on scalar/vector. The two
                # heads' concurrent row-tiled matmuls hit different banks
                # (A at col 0, B at col 512).
                for j in (2 * g, 2 * g + 1):
                    psS = attn_sp.tile([128, 1024], F32, tag="s")
                    pt = pt_pool.tile([128, 1024], CD, tag="pt")
                    off = max(0, 128 * j - 512 * ib)
                    w = 512 - off
                    cols = [0, 512]
                    for half in range(2):
                        nc.tensor.matmul(
                            out=psS[:, cols[half]:cols[half] + w],
                            lhsT=kpair[p][64 * half:64 * half + 64,
                                          j * 128:(j + 1) * 128],
                            rhs=qt[64 * half:64 * half + 64,
                                   ib * 512 + off:(ib + 1) * 512],
                            start=True,
                            stop=True,
                        )
                    cur.append((psS, pt, j, cols, off))
                done = []
                for (psS, pt, j, cols, off) in cur:
                    w = 512 - off
                    # one exp over both banks via a strided 2-region AP
                    # ([128, 2, w]) - no wasted columns on diagonal tiles
                    nc.scalar.activation(
                        out=pt[:].rearrange("p (b c) -> p b c", b=2)[:, :, 0:w],
                        in_=psS[:].rearrange("p (b c) -> p b c", b=2)[:, :, 0:w],
                        func=Exp,
                        scale=0.125,
                    )
                    if 128 * j >= 512 * ib:  # diagonal tile -> mask both
                        for c in cols:
                            nc.vector.tensor_mul(
                                pt[:, c:c + 128], pt[:, c:c + 128], tri[:],
                            )
                    done.append((pt, j, cols, off))
                if prev:
                    emit_pv(prev, last=False)
                prev = done
            emit_pv(prev, last=True)
            if not last_pair:
                lrows, lbcs = [], []
                for half in range(2):
                    lrow = misc.tile([1, 512], F32, tag="lrow")
                    nc.vector.tensor_copy(lrow[:], psy[half][HS:HS + 1, :])
                    lrows.append(lrow)
                for half in range(2):
                    linv = misc.tile([1, 512], F32, tag="linv")
                    nc.vector.reciprocal_approx_fast(linv[:], lrows[half][:])
                    lbc = misc.tile([64, 512], F32, tag="lbc")
                    nc.gpsimd.partition_broadcast(
                        lbc[:], linv[:], channels=64
                    )
                    lbcs.append(lbc)
                for half in range(2):
                    nc.vector.tensor_mul(
                        yt_s[half * 64:half * 64 + 64, p, isl],
                        psy[half][0:HS, :], lbcs[half][:]
                    )
                continue

            # last pair: bank-X normalize overlaps the bank-Y final PVs;
            # proj t-tiles 12/13 (X columns) overlap the bank-Y normalize.
            # One recip+broadcast covers both heads (their l-rows packed
            # side by side), halving the serial chain.
            def norm_bank(bank, yt_c0):
                lrow = misc.tile([1, 512], F32, tag="lrow")
                nc.scalar.copy(lrow[:, 0:256], bank[HS:HS + 1, 0:256])
                nc.vector.tensor_copy(lrow[:, 256:512],
                                      bank[HS:HS + 1, 256:512])
                linv = misc.tile([1, 512], F32, tag="linv")
                nc.vector.reciprocal_approx_fast(linv[:], lrow[:])
                lbc = misc.tile([64, 512], F32, tag="lbc")
                nc.gpsimd.partition_broadcast(lbc[:], linv[:], channels=64)
                for half in range(2):
                    s = half * 256
                    nc.vector.tensor_mul(
                        yt_s[half * 64:half * 64 + 64, p,
                             yt_c0:yt_c0 + 256],
                        bank[0:HS, s:s + 256], lbc[:, s:s + 256]
                    )

            base = ib * 512
            norm_bank(psyX, base)
            norm_bank(psyY, base + 256)
            for tloc in range(4):
                emit_proj_tloc_tail(ib * 4 + tloc)

    # braid plan: ib=3 is ACT-bound so it gets tb3's k/v units (not needed
    # until its j>=12 tiles) plus proj(1) and proj(2). fills_per_window=2
    # in ib0 (few windows) and ib3 (emission of k units must stay ahead of
    # the j=12 S matmul in window 7 of head-pair 0).
    fq = []
    fill_sched = [[1]]
    fill_idx = [0]

    def fill():
        sched = fill_sched[0]
        n = sched[min(fill_idx[0], len(sched) - 1)]
        fill_idx[0] += 1
        for _ in range(n):
            if fq:
                fq.pop(0)()

    emit_ph1_block0()
    plan = {
        0: ph1_units(1, "all"),
        1: ph1_units(2, "kq") + proj_units(0),
        2: ph1_units(2, "v") + ph1_units(3, "q") + proj_units(1),
        3: ph1_units(3, "k") + ph1_units(3, "v") + proj_units(2),
    }
    # pops per window, front-loaded to meet emission deadlines (k/v units
    # before the S/PV that consume them) but spread so the ACT-bound late
    # ib3 windows keep the PE fed (HAM re-throttles after ~3.4us of low
    # PE activity, halving the clock for everything that follows)
    fills = {
        0: [2],
        1: [1, 1, 1, 1, 2, 2, 2, 2],
        2: [1, 1, 1, 1, 2, 2, 2, 2, 2, 1, 1, 1],
        3: [2, 2, 2, 2, 1, 1, 1, 1, 1, 1, 1, 1, 1, 0, 0, 0],
    }
    for ib in range(IB):
        fq.extend(plan[ib])
        fill_sched[0] = fills[ib]
        fill_idx[0] = 0
        emit_attn_block(ib, fill)
        while fq:
            fq.pop(0)()


def build():
    if "nc" in _CACHE:
        return _CACHE["nc"]
    nc = bacc.Bacc(
        "TRN2", target_bir_lowering=False, debug=False, num_devices=N_CORES
    )
    aps = {
        "xt": nc.dram_tensor("xt", [128, KT, T], CD, kind="ExternalInput").ap(),
        "wqkv": nc.dram_tensor("wqkv", [128, KT, 6, 128], CD, kind="ExternalInput").ap(),
        "bqk": nc.dram_tensor("bqk", [128, 4], F32, kind="ExternalInput").ap(),
        "bv": nc.dram_tensor("bv", [1, 256], F32, kind="ExternalInput").ap(),
        "wp": nc.dram_tensor("wp", [128, 2, C], CD, kind="ExternalInput").ap(),
        "out": nc.dram_tensor("out", [T, C], CD, kind="ExternalOutput").ap(),
    }
    from contextlib import ExitStack

    with tile.TileContext(nc) as tc:
        with ExitStack() as ctx:
            _emit(nc, tc, ctx, aps)
    nc.compile()
    _CACHE["nc"] = nc
    return nc


def make_in_maps(x, W_attn, b_attn, W_proj, b_proj):
    x = np.asarray(x, dtype=np.float32)
    W_attn = np.asarray(W_attn, dtype=np.float32)
    b_attn = np.asarray(b_attn, dtype=np.float32)
    W_proj = np.asarray(W_proj, dtype=np.float32)

    in_maps = []
    xt_b = {}
    for b in range(B):
        xt = np.ascontiguousarray(x[b].T)  # [C, T]
        xt_b[b] = (
            xt.reshape(KT, 128, T).transpose(1, 0, 2).astype(CD_NP)
        )
    for core in range(N_CORES):
        b = core // 4
        g = core % 4
        fs = slice(256 * g, 256 * g + 256)  # feature cols for this head group
        wq = W_attn[:, fs]
        wk = W_attn[:, C + 256 * g: C + 256 * g + 256]
        wv = W_attn[:, 2 * C + 256 * g: 2 * C + 256 * g + 256]
        wqkv = np.concatenate([wq, wk, wv], axis=1)  # [1024, 768]
        bq = b_attn[fs]
        bk = b_attn[C + 256 * g: C + 256 * g + 256]
        bv = b_attn[2 * C + 256 * g: 2 * C + 256 * g + 256]
        in_maps.append({
            "xt": xt_b[b],
            "wqkv": np.ascontiguousarray(
                wqkv.reshape(KT, 128, 6, 128).transpose(1, 0, 2, 3)
            ).astype(CD_NP),
            "bqk": np.ascontiguousarray(
                np.concatenate([bq, bk]).reshape(4, 128).T
            ).astype(np.float32),
            "bv": bv[None, :].astype(np.float32),
            "wp": np.ascontiguousarray(
                W_proj[fs, :].reshape(2, 128, C).transpose(1, 0, 2)
            ).astype(CD_NP),
        })
    return in_maps


def _ensure_ntff_hook():
    """Recreate the missing antenv.axon_hooks NTFF-profile shim (see
    trn_agent_boot/trn_boot.py) so run_bass_kernel_spmd(trace=True) works."""
    import contextlib
    import ctypes
    import types

    try:
        from antenv.axon_hooks import get_axon_ntff_profile_hook  # noqa: F401

        return
    except ImportError:
        pass

    mod = types.ModuleType("antenv.axon_hooks")
    _holder = {"hook": None}
    mod.set_axon_ntff_profile_hook = lambda h: _holder.__setitem__("hook", h)
    mod.get_axon_ntff_profile_hook = lambda: _holder["hook"]
    sys.modules["antenv.axon_hooks"] = mod
    import antenv

    antenv.axon_hooks = mod

    so_path = "/opt/axon/libaxon_pjrt.so"
    if not os.path.exists(so_path):
        return
    lib = ctypes.CDLL(so_path)
    if not hasattr(lib, "axon_start_nrt_profile"):
        return
    lib.axon_start_nrt_profile.argtypes = [
        ctypes.POINTER(ctypes.c_int64),
        ctypes.c_size_t,
    ]
    lib.axon_start_nrt_profile.restype = ctypes.c_int64
    lib.axon_stop_nrt_profile.argtypes = [ctypes.c_char_p]
    lib.axon_stop_nrt_profile.restype = ctypes.c_int64

    @contextlib.contextmanager
    def _hook(output_dir, device_ids):
        import jax

        jax.devices()
        if device_ids:
            ids = (ctypes.c_int64 * len(device_ids))(*device_ids)
            rc = lib.axon_start_nrt_profile(ids, len(device_ids))
        else:
            rc = lib.axon_start_nrt_profile(None, 0)
        if rc != 0:
            raise RuntimeError(f"axon_start_nrt_profile rc={rc}")
        try:
            yield
        finally:
            n = lib.axon_stop_nrt_profile(str(output_dir).encode())
            if n <= 0:
                print(f"ntff profile: rc={n}, nothing written to {output_dir}")

    mod.set_axon_ntff_profile_hook(_hook)


def kernel(x, W_attn, b_attn, W_proj, b_proj):
    global LAST_RESULT
    nc = build()
    in_maps = make_in_maps(x, W_attn, b_attn, W_proj, b_proj)
    b_proj = np.asarray(b_proj, dtype=np.float32)
    trace = os.environ.get("KERNEL_TRACE", "0") == "1"
    if trace:
        _ensure_ntff_hook()
        import concourse.bass_utils as _bu

        _bu.upload_artifacts = lambda tmpdir: f"local://{tmpdir}"
    res = run_bass_kernel_spmd(
        nc, in_maps, core_ids=list(range(N_CORES)), trace=trace
    )
    LAST_RESULT = res
    outs = [res.results[i]["out"] for i in range(N_CORES)]
    y = np.empty((B, T, C), dtype=np.float32)
    for b in range(B):
        acc = outs[4 * b].astype(np.float32)
        for g in range(1, 4):
            acc = acc + outs[4 * b + g].astype(np.float32)
        y[b] = acc + b_proj
    return y
